# revision 29
# baseline (speedup 1.0000x reference)
"""Trainium2 Bass kernel for nn_CLGODE (graph coupled latent graph ODE).

Strategy (pure data parallel, 2 samples per core x 8 cores):
  - Host precomputes: normalized-adjacency pieces (a_hat, d, d*x0), covariate
    embeddings folded with the time term into a per-(sample,eval) bias table,
    h*W2 folded weights per (sample,interval), and all small constants.
  - Device (per core, feature-major state zT [128, 2, 400]):
      encoder GCN + 2 MLP heads -> z0
      RK4 ODE: 7 intervals x 8 steps x 4 stages; stage MLPs on PE,
      relu+bias fused PSUM->SBUF on ACT/DVE, RK4 combines as fused
      scalar_tensor_tensor ops on DVE.
      decoder: score = zc zc^T via PE (symmetric => row blocks only),
      softplus/affine on ACT/DVE, x_hat MLP.
  - Outputs written feature-major where needed; host does the cheap
    final transposes (x_hat, zm, zc, mu/lv). logit/w/l_new are symmetric
    and written directly.
"""
import numpy as np
import ml_dtypes

import concourse.bass as bass
import concourse.bacc as bacc
import concourse.mybir as mybir
import concourse.tile as tile
import concourse.bass_utils as bass_utils

F32 = mybir.dt.float32
BF16 = mybir.dt.bfloat16
AF = mybir.ActivationFunctionType
OP = mybir.AluOpType


def _patch_act_tables():
    """Prefer the table set holding exp+ln+relu+identity+copy so the whole
    kernel needs a single ACT_TABLE_LOAD instead of thrashing per call."""
    import concourse.bacc as _bm
    if getattr(_bm, "_act_tables_patched", False):
        return
    orig = _bm.get_activation_tables

    # IMPORTANT: the first entry of the table list is treated as the
    # boot-resident set (no load emitted before its first use), so the
    # original order must be preserved — reordering makes ops run against
    # the wrong resident tables (observed: Ln returning garbage). With the
    # original order, Exp loads exp_and_others and Ln loads natural_log;
    # the batched score phase keeps that to ~2 loads per time-point.
    _bm._act_tables_patched = True

# problem constants (hardcoded per contract)
B, N, MD, L, H, CE, NCOV, STEPS, T = 16, 400, 8, 64, 128, 8, 2, 8, 8
NIV = T - 1                       # 7 intervals
ZD = 2 * L                        # 128 latent dims
NCORES = 8
S = B // NCORES                   # 2 samples per core
JB = [(0, 128), (128, 128), (256, 128), (384, 16)]  # node row blocks
RK_C = [0.5, 0.5, 1.0]            # stage input coefficients
RK_W = [1.0 / 6, 1.0 / 3, 1.0 / 3, 1.0 / 6]

_CACHE = {}


def _beff_col(iv, k, st, path, s):
    return (((iv * STEPS + k) * 4 + st) * 2 + path) * 2 + s


def _build(scalars, use_b2fix, use_lnew):
    """Build + compile the (SPMD, per-core) Bass program once."""
    alpha, delta, gamma, beta, alpha_new, delta_new = scalars
    nc = bacc.Bacc("TRN2", target_bir_lowering=False, debug=False,
                   num_devices=NCORES)

    def din(name, shape):
        return nc.dram_tensor(name, shape, F32, kind="ExternalInput").ap()

    def dout(name, shape):
        return nc.dram_tensor(name, shape, F32, kind="ExternalOutput").ap()

    def dinb(name, shape):
        return nc.dram_tensor(name, shape, BF16, kind="ExternalInput").ap()

    ahat_d = din("ahat", [S, N, N])
    u_d = din("u", [S, N, MD])
    dvec_d = din("dvec", [S, N])
    w1z_d = dinb("w1z", [2, ZD, H])
    hbw2_d = dinb("hbw2", [NIV, S, 2, H, L])
    beff_d = din("beff", [ZD, NIV * STEPS * 4 * 4])
    encw1_d = din("encw1", [2, MD, H])
    wmu_d = din("wmu", [2, H, L])
    wlv_d = din("wlv", [2, H, L])
    dmw1_d = din("dmw1", [L, H])
    dmw2_d = din("dmw2", [H, MD])
    encb_d = din("encb", [H, 6])
    ident_d = din("ident", [128, 128])
    masks_d = din("masks", [128, 4 * N])
    if use_b2fix:
        b2fix_d = din("b2fix", [1, NIV * S * ZD])

    logit_d = dout("logit", [S, T, N, N])
    w_d = dout("wout", [S, T, N, N])
    if use_lnew:
        lnew_d = dout("lnew", [S, T, N, N])
    xhatT_d = dout("xhatT", [S, T, MD, N])
    zt_d = dout("zt", [T, S, ZD, N])
    warm_d = dout("warm", [128, 512])
    muT_d = dout("muT", [S, ZD, N])
    lvT_d = dout("lvT", [S, ZD, N])

    with tile.TileContext(nc) as tc:
        from contextlib import ExitStack
        with ExitStack() as ctx:
            const = ctx.enter_context(tc.tile_pool(name="const", bufs=1))
            # ---- constants into SBUF ----
            w1z_sb = const.tile([ZD, 2, H], BF16, name="w1z_sb")
            nc.sync.dma_start(w1z_sb[:], w1z_d.rearrange("p k m -> k p m"))
            hbw2_sb = const.tile([H, NIV, S, 2, L], BF16, name="hbw2_sb")
            nc.sync.dma_start(hbw2_sb[:], hbw2_d.rearrange("i s p k m -> k i s p m"))
            beff_sb = const.tile([ZD, NIV * STEPS * 4 * 4], F32, name="beff_sb")
            nc.sync.dma_start(beff_sb[:], beff_d)
            encw1_sb = const.tile([MD, 2, H], F32, name="encw1_sb")
            nc.sync.dma_start(encw1_sb[:], encw1_d.rearrange("p k m -> k p m"))
            wmu_sb = const.tile([H, 2, L], F32, name="wmu_sb")
            nc.sync.dma_start(wmu_sb[:], wmu_d.rearrange("p k m -> k p m"))
            wlv_sb = const.tile([H, 2, L], F32, name="wlv_sb")
            nc.sync.dma_start(wlv_sb[:], wlv_d.rearrange("p k m -> k p m"))
            dmw1_sb = const.tile([L, H], F32, name="dmw1_sb")
            nc.sync.dma_start(dmw1_sb[:], dmw1_d)
            dmw2_sb = const.tile([H, MD], F32, name="dmw2_sb")
            nc.sync.dma_start(dmw2_sb[:], dmw2_d)
            encb_sb = const.tile([H, 6], F32, name="encb_sb")
            nc.sync.dma_start(encb_sb[:], encb_d)
            ident_sb = const.tile([128, 128], F32, name="ident_sb")
            nc.sync.dma_start(ident_sb[:], ident_d)
            masks_sb = const.tile([128, 4 * N], F32, name="masks_sb")
            nc.sync.dma_start(masks_sb[:], masks_d)
            if use_b2fix:
                b2fix_sb = const.tile([1, NIV * S * ZD], F32, name="b2fix_sb")
                nc.sync.dma_start(b2fix_sb[:], b2fix_d)
                ones_sb = const.tile([1, N], F32, name="ones_sb")
                nc.vector.memset(ones_sb[:], 1.0)

            # ---- persistent z history + work tiles ----
            zhp = ctx.enter_context(tc.tile_pool(name="zhp", bufs=1))
            zh = [zhp.tile([ZD, S, 512], F32, tag=f"zh{t}", name=f"zh{t}")
                  for t in range(T)]
            work = ctx.enter_context(tc.tile_pool(name="work", bufs=2))

            # ---- psum pools: 2+2+2+2 = 8 banks ----
            pAm = ctx.enter_context(tc.tile_pool(name="pAm", bufs=2, space="PSUM"))
            pAc = ctx.enter_context(tc.tile_pool(name="pAc", bufs=2, space="PSUM"))
            pC = ctx.enter_context(tc.tile_pool(name="pC", bufs=1, space="PSUM"))
            pM = ctx.enter_context(tc.tile_pool(name="pM", bufs=2, space="PSUM"))

            stage = ctx.enter_context(tc.tile_pool(name="stage", bufs=3))
            use_gp_lg2 = (gamma * delta + beta) == 0.0
            if use_gp_lg2:
                # w-chain: spu = (g/a)*logit + log1p-part (exact when g*d+b=0)
                csc_sb = const.tile([128, N], F32, name="csc_sb")
                nc.vector.memset(csc_sb[:], float(gamma / alpha))

            # ---- PE warm-up: ~11us dense matmul burst flips the HAM clock
            # gate to K=8/8 before the encoder; consumed via a dummy output.
            wps = pM.tile([128, 512], F32, tag="m", name="wps")
            for i in range(26):
                nc.tensor.matmul(wps[:, 0:512], w1z_sb[:, 0, :],
                                 hbw2_sb[:, 0:2, :, :, :],
                                 start=(i == 0), stop=(i == 25))
            wsb = stage.tile([128, 512], F32, tag="warm", name="wsb")
            nc.scalar.copy(wsb[:], wps[:, 0:512])
            nc.sync.dma_start(warm_d[:], wsb[:])


            # ================= encoder =================
            with tc.tile_pool(name="enc", bufs=1) as enc:
                ahat_sb = []
                for s in range(S):
                    a = enc.tile([128, 4, N], F32, tag=f"ah{s}", name=f"ah{s}")
                    nc.sync.dma_start(
                        a[:, 0:3, :],
                        ahat_d[s, 0:384].rearrange("(k p) n -> p k n", p=128))
                    nc.sync.dma_start(a[0:16, 3, :], ahat_d[s, 384:400])
                    ahat_sb.append(a)
                u_sb = enc.tile([128, S, 4, MD], F32, name="u_sb")
                dv_sb = enc.tile([128, S, 4], F32, name="dv_sb")
                for s in range(S):
                    nc.sync.dma_start(
                        u_sb[:, s, 0:3, :],
                        u_d[s, 0:384].rearrange("(k p) d -> p k d", p=128))
                    nc.sync.dma_start(u_sb[0:16, s, 3, :], u_d[s, 384:400])
                    nc.sync.dma_start(
                        dv_sb[:, s, 0:3],
                        dvec_d[s, 0:384].rearrange("(k p) -> p k", p=128))
                    nc.sync.dma_start(dv_sb[0:16, s, 3], dvec_d[s, 384:400])

                hx_sb = enc.tile([128, S, 4, MD], F32, name="hx_sb")
                hxT_sb = enc.tile([MD, S, 512], F32, name="hxT_sb")
                for s in range(S):
                    for j, (j0, jsz) in enumerate(JB):
                        ps_y = pM.tile([128, 512], F32, tag="m", name="ps_y")
                        for kb, (k0, ksz) in enumerate(JB):
                            nc.tensor.matmul(
                                ps_y[0:jsz, 0:MD],
                                ahat_sb[s][0:ksz, kb, j0:j0 + jsz],
                                u_sb[0:ksz, s, kb, :],
                                start=(kb == 0), stop=(kb == 3))
                        nc.vector.tensor_scalar(
                            hx_sb[0:jsz, s, j, :], ps_y[0:jsz, 0:MD],
                            dv_sb[0:jsz, s, j:j + 1], None, OP.mult)
                        ps_hxT = pM.tile([128, 512], F32, tag="m", name="ps_hxT")
                        nc.tensor.transpose(
                            ps_hxT[0:MD, 0:jsz], hx_sb[0:jsz, s, j, :],
                            ident_sb[0:jsz, 0:jsz])
                        nc.scalar.copy(hxT_sb[:, s, j0:j0 + jsz],
                                       ps_hxT[0:MD, 0:jsz])

                hm_sb = enc.tile([H, S, 512], F32, name="hm_sb")
                hc_sb = enc.tile([H, S, 512], F32, name="hc_sb")
                for s in range(S):
                    ps_hm = pAm.tile([H, 512], F32, tag="a", name="ps_hm")
                    nc.tensor.matmul(ps_hm[:, 0:N], encw1_sb[:, 0, :],
                                     hxT_sb[:, s, 0:N])
                    nc.scalar.activation(hm_sb[:, s, 0:N], ps_hm[:, 0:N],
                                         AF.Relu, bias=encb_sb[:, 0:1])
                    ps_hc = pAc.tile([H, 512], F32, tag="a", name="ps_hc")
                    nc.tensor.matmul(ps_hc[:, 0:N], encw1_sb[:, 1, :],
                                     hxT_sb[:, s, 0:N])
                    nc.scalar.activation(hc_sb[:, s, 0:N], ps_hc[:, 0:N],
                                         AF.Relu, bias=encb_sb[:, 1:2])
                lv_sb = enc.tile([ZD, S, 512], F32, name="lv_sb")
                for s in range(S):
                    ps_mu = pC.tile([ZD, S, 512], F32, tag="c", name="ps_mu")
                    nc.tensor.matmul(ps_mu[0:L, s, 0:N], wmu_sb[:, 0, :],
                                     hm_sb[:, s, 0:N])
                    nc.tensor.matmul(ps_mu[L:ZD, s, 0:N], wmu_sb[:, 1, :],
                                     hc_sb[:, s, 0:N])
                    nc.scalar.activation(zh[0][:, s, 0:N], ps_mu[:, s, 0:N],
                                         AF.Identity, bias=encb_sb[:, 2:3])
                    nc.sync.dma_start(muT_d[s], zh[0][:, s, 0:N])
                    ps_lv = pM.tile([ZD, 512], F32, tag="m", name="ps_lv")
                    nc.tensor.matmul(ps_lv[0:L, 0:N], wlv_sb[:, 0, :],
                                     hm_sb[:, s, 0:N])
                    nc.tensor.matmul(ps_lv[L:ZD, 0:N], wlv_sb[:, 1, :],
                                     hc_sb[:, s, 0:N])
                    nc.scalar.activation(lv_sb[:, s, 0:N], ps_lv[:, 0:N],
                                         AF.Identity, bias=encb_sb[:, 3:4])
                    nc.sync.dma_start(lvT_d[s], lv_sb[:, s, 0:N])

            # ================= decode/score phase =================
            # Batched per t: all exps back-to-back, then all lns, so the ACT
            # table set switches only twice per t. Post-ln arithmetic runs on
            # GpSimd using the logit tensor (score = (logit+a*d)/a).
            def score_phase(t):
                lgt, lg2t, spet, spt = {}, {}, {}, {}
                zbt = {}
                for s in range(S):
                    zb = stage.tile([ZD, 512], BF16, tag=f"zb{s}", bufs=2,
                                    name="zb")
                    nc.vector.tensor_copy(zb[:, 0:N], zh[t][:, s, 0:N])
                    zbt[s] = zb
                for s in range(S):
                    for j, (j0, jsz) in enumerate(JB):
                        ps_sc = pM.tile([128, 512], F32, tag="m", name="ps_sc")
                        nc.tensor.matmul(ps_sc[0:jsz, 0:N],
                                         zbt[s][L:ZD, j0:j0 + jsz],
                                         zbt[s][L:ZD, 0:N])
                        lg = stage.tile([128, N], F32, tag=f"lg{s}{j}",
                                        bufs=1, name="lg")
                        nc.scalar.activation(
                            lg[0:jsz, :], ps_sc[0:jsz, 0:N], AF.Copy,
                            bias=float(-alpha * delta), scale=float(alpha))
                        nc.sync.dma_start(logit_d[s, t, j0:j0 + jsz, :],
                                          lg[0:jsz, :])
                        lg2 = stage.tile([128, N], F32, tag=f"lg2{s}{j}",
                                         bufs=1, name="lg2")
                        if use_gp_lg2:
                            nc.gpsimd.tensor_tensor(
                                lg2[0:jsz, :], lg[0:jsz, :],
                                csc_sb[0:jsz, :], OP.mult)
                        else:
                            nc.vector.tensor_scalar(
                                lg2[0:jsz, :], ps_sc[0:jsz, 0:N],
                                float(gamma), float(beta), OP.mult, OP.add)
                        if use_lnew:
                            lnw = stage.tile([128, N], F32, tag="lnw",
                                             name="lnw")
                            nc.vector.tensor_scalar(
                                lnw[0:jsz, :], ps_sc[0:jsz, 0:N],
                                float(alpha_new), float(-alpha_new * delta_new),
                                OP.mult, OP.add)
                            nc.sync.dma_start(lnew_d[s, t, j0:j0 + jsz, :],
                                              lnw[0:jsz, :])
                        # stable softplus(x)=x+log1p(exp(-x)), x=g*s+b >= 0
                        spe = stage.tile([128, N], F32, tag=f"spe{s}{j}",
                                         bufs=1, name="spe")
                        exp_inst = nc.scalar.activation(
                            spe[0:jsz, :], ps_sc[0:jsz, 0:N],
                            AF.Exp, bias=float(-beta), scale=float(-gamma))
                        lgt[(s, j)] = lg
                        lg2t[(s, j)] = lg2
                        spet[(s, j)] = spe
                from concourse.tile import add_dep_helper
                for s in range(S):
                    for j, (j0, jsz) in enumerate(JB):
                        sp = stage.tile([128, N], F32, tag=f"sp{s}{j}",
                                        bufs=1, name="sp")
                        ln_inst = nc.scalar.activation(
                            sp[0:jsz, :], spet[(s, j)][0:jsz, :],
                            AF.Ln, bias=1.0)
                        # whole ln batch after whole exp batch: 2 ACT table
                        # switches per t instead of ~5
                        add_dep_helper(ln_inst.ins, exp_inst.ins, sync=True,
                                       reason="batch ln after exp")
                        spt[(s, j)] = sp
                for s in range(S):
                    for j, (j0, jsz) in enumerate(JB):
                        spu = stage.tile([128, N], F32, tag="spu", name="spu")
                        nc.gpsimd.tensor_tensor(
                            spu[0:jsz, :], lg2t[(s, j)][0:jsz, :],
                            spt[(s, j)][0:jsz, :], OP.add)
                        wt = stage.tile([128, N], F32, tag="wt", name="wt")
                        nc.gpsimd.tensor_tensor(
                            wt[0:jsz, :], spu[0:jsz, :],
                            masks_sb[0:jsz, j * N:(j + 1) * N], OP.mult)
                        nc.sync.dma_start(w_d[s, t, j0:j0 + jsz, :],
                                          wt[0:jsz, :])
                for s in range(S):
                    # x_hat head
                    ps_hd = pM.tile([128, 512], F32, tag="m", name="ps_hd")
                    nc.tensor.matmul(ps_hd[:, 0:N], dmw1_sb[:],
                                     zh[t][0:L, s, 0:N])
                    hd = stage.tile([H, N], F32, tag="hd", name="hd")
                    nc.scalar.activation(hd[:], ps_hd[:, 0:N], AF.Relu,
                                         bias=encb_sb[:, 4:5])
                    ps_xh = pM.tile([128, 512], F32, tag="m", name="ps_xh")
                    nc.tensor.matmul(ps_xh[0:MD, 0:N], dmw2_sb[:], hd[:])
                    xh = stage.tile([MD, N], F32, tag="xh", name="xh")
                    nc.scalar.activation(xh[:], ps_xh[0:MD, 0:N], AF.Identity,
                                         bias=encb_sb[0:MD, 5:6])
                    nc.sync.dma_start(xhatT_d[s, t], xh[:])
                    nc.sync.dma_start(zt_d[t, s], zh[t][:, s, 0:N])

            # ================= one RK4 step =================
            def ode_step(iv, k, z_in, z_out):
                acc = z_in
                zbf = work.tile([ZD, S, 512], BF16, tag="zbf", name="zbf")
                nc.vector.tensor_copy(zbf[:, :, 0:N], z_in[:, :, 0:N])
                ytile = zbf
                for st in range(4):
                    ev = (iv * STEPS + k) * 4 + st
                    psA = [[pAm.tile([H, 512], F32, tag="a", name="psAm")
                            for s in range(S)],
                           [pAc.tile([H, 512], F32, tag="a", name="psAc")
                            for s in range(S)]]
                    for p in range(2):
                        # pacemaker: dep-free filler that issues as soon as
                        # the psum slot frees, keeping the PE HAM clock gate
                        # warm through the relu/STT wait; overwritten by the
                        # real matmul below (start=True).
                        nc.tensor.matmul(psA[p][0][:, 0:384],
                                         w1z_sb[:, p, :],
                                         hbw2_sb[:, 0:3, :, 0, :])
                    for p in range(2):
                        for s in range(S):
                            nc.tensor.matmul(psA[p][s][:, 0:N],
                                             w1z_sb[:, p, :],
                                             ytile[:, s, 0:N])
                    h1 = [work.tile([H, S, 512], BF16, tag="h1m", name="h1m"),
                          work.tile([H, S, 512], BF16, tag="h1c", name="h1c")]
                    for p in range(2):
                        for s in range(S):
                            r = st * 4 + p * 2 + s
                            bcol = _beff_col(iv, k, st, p, s)
                            bias_ap = beff_sb[:, bcol:bcol + 1]
                            if r % 8 == 3:   # 2 of 16 per step on DVE
                                nc.vector.tensor_scalar(
                                    h1[p][:, s, 0:N], psA[p][s][:, 0:N],
                                    bias_ap, 0.0, OP.add, OP.max)
                            else:
                                nc.scalar.activation(
                                    h1[p][:, s, 0:N], psA[p][s][:, 0:N],
                                    AF.Relu, bias=bias_ap)
                    psC = pC.tile([ZD, S, 512], F32, tag="c", name="psC")
                    nc.tensor.matmul(psC[:, 0, 0:384], w1z_sb[:, 1, :],
                                     hbw2_sb[:, 3:6, :, 0, :])
                    fix = use_b2fix and st == 3
                    for s in range(S):
                        if fix:
                            off = (iv * S + s) * ZD
                            nc.tensor.matmul(
                                psC[:, s, 0:N],
                                b2fix_sb[:, off:off + ZD],
                                ones_sb[:, 0:N], start=True, stop=False)
                        for p in range(2):
                            nc.tensor.matmul(
                                psC[p * L:(p + 1) * L, s, 0:N],
                                hbw2_sb[:, iv, s, p, :],
                                h1[p][:, s, 0:N],
                                start=not fix, stop=True)
                    if st < 3:
                        ynew = work.tile([ZD, S, 512], BF16, tag="y", name="y")
                        for s in range(S):
                            nc.vector.scalar_tensor_tensor(
                                ynew[:, s, 0:N], psC[:, s, 0:N], RK_C[st],
                                z_in[:, s, 0:N], OP.mult, OP.add)
                        accnew = work.tile([ZD, S, 512], F32, tag="acc",
                                           name="acc")
                        nc.vector.scalar_tensor_tensor(
                            accnew[:, :, 0:N], psC[:, :, 0:N], RK_W[st],
                            acc[:, :, 0:N], OP.mult, OP.add)
                        ytile = ynew
                        acc = accnew
                    else:
                        nc.vector.scalar_tensor_tensor(
                            z_out[:, :, 0:N], psC[:, :, 0:N], RK_W[st],
                            acc[:, :, 0:N], OP.mult, OP.add)

            # ================= main schedule =================
            score_phase(0)
            for iv in range(NIV):
                z_in = zh[iv]
                for k in range(STEPS):
                    if k == STEPS - 1:
                        z_out = zh[iv + 1]
                    else:
                        z_out = work.tile([ZD, S, 512], F32, tag="zw",
                                          name="zw")
                    ode_step(iv, k, z_in, z_out)
                    z_in = z_out
                score_phase(iv + 1)

    _patch_act_tables()
    nc.compile()
    return nc


def _host_prep(inputs, core):
    """Per-core input arrays (numpy) for in_maps."""
    f32 = np.float32
    bsel = [core * S + s for s in range(S)]
    a0 = inputs["a0"][bsel].astype(np.float64)
    x0 = inputs["x0"][bsel].astype(np.float64)
    times = inputs["times"].astype(np.float64)
    eye = np.eye(N)
    ahat = a0 + eye
    d = (ahat.sum(-1) + 1e-8) ** -0.5            # [S,N]
    u = d[..., None] * x0                        # [S,N,MD]

    sex = np.asarray(inputs["sex"]).astype(np.int64)
    site = np.asarray(inputs["site"]).astype(np.int64)
    cov_full = np.concatenate([
        np.asarray(inputs["sex_emb"])[sex],
        np.asarray(inputs["site_emb"])[site],
        np.asarray(inputs["covariates"])], -1).astype(np.float64)  # [B, 18]

    omW1 = np.asarray(inputs["om_W1"], np.float64)
    ocW1 = np.asarray(inputs["oc_W1"], np.float64)
    omW2 = np.asarray(inputs["om_W2"], np.float64)
    ocW2 = np.asarray(inputs["oc_W2"], np.float64)
    omb1 = np.asarray(inputs["om_b1"], np.float64)
    ocb1 = np.asarray(inputs["oc_b1"], np.float64)
    omb2 = np.asarray(inputs["om_b2"], np.float64)
    ocb2 = np.asarray(inputs["oc_b2"], np.float64)
    b2cat = np.concatenate([omb2, ocb2])          # [128]
    W1z = np.stack([omW1[:ZD], ocW1[:ZD]])        # [2,128,128]
    W1cov = np.stack([omW1[ZD:ZD + 18], ocW1[ZD:ZD + 18]])  # [2,18,128]
    W1t = np.stack([omW1[ZD + 18], ocW1[ZD + 18]])          # [2,128]
    b1 = np.stack([omb1, ocb1])

    hbw2 = np.zeros([NIV, S, 2, H, L])
    beff = np.zeros([ZD, NIV * STEPS * 4 * 4])
    b2fix = np.zeros([1, NIV * S * ZD])
    for s in range(S):
        b = bsel[s]
        base = [cov_full[b] @ W1cov[p] + b1[p] for p in range(2)]
        for iv in range(NIV):
            t0 = times[b, iv]
            hh = (times[b, iv + 1] - t0) / STEPS
            hb = hh
            hbw2[iv, s, 0] = hb * omW2
            hbw2[iv, s, 1] = hb * ocW2
            b2fix[0, (iv * S + s) * ZD:(iv * S + s + 1) * ZD] = 6.0 * hb * b2cat
            for k in range(STEPS):
                tk = t0 + k * hh
                toff = [tk, tk + 0.5 * hh, tk + 0.5 * hh, tk + hh]
                for st in range(4):
                    for p in range(2):
                        v = base[p] + toff[st] * W1t[p]
                        if st > 0:
                            # y_st misses c*(hb*b2cat): pre-act correction
                            v = v + (RK_C[st - 1] * hb) * (b2cat @ W1z[p])
                        beff[:, _beff_col(iv, k, st, p, s)] = v

    masks = np.ones([128, 4 * N], f32)
    for j, (j0, jsz) in enumerate(JB):
        for p in range(jsz):
            masks[p, j * N + j0 + p] = 0.0

    def c(a):
        return np.ascontiguousarray(a, dtype=f32)

    m = {
        "ahat": c(ahat), "u": c(u), "dvec": c(d),
        "w1z": np.ascontiguousarray(W1z, dtype=ml_dtypes.bfloat16),
        "hbw2": np.ascontiguousarray(hbw2, dtype=ml_dtypes.bfloat16),
        "beff": c(beff),
        "encw1": c(np.stack([inputs["em_W1"], inputs["ec_W1"]])),
        "wmu": c(np.stack([inputs["em_Wmu"], inputs["ec_Wmu"]])),
        "wlv": c(np.stack([inputs["em_Wlv"], inputs["ec_Wlv"]])),
        "dmw1": c(inputs["dm_W1"]), "dmw2": c(inputs["dm_W2"]),
        "ident": c(np.eye(128)),
        "masks": masks,
    }
    encb = np.zeros([H, 6])
    encb[:, 0] = inputs["em_b1"]
    encb[:, 1] = inputs["ec_b1"]
    encb[:, 2] = np.concatenate([inputs["em_bmu"], inputs["ec_bmu"]])
    encb[:, 3] = np.concatenate([inputs["em_blv"], inputs["ec_blv"]])
    encb[:, 4] = inputs["dm_b1"]
    encb[0:MD, 5] = inputs["dm_b2"]
    m["encb"] = c(encb)
    if np.abs(b2cat).max() > 0:
        m["b2fix"] = c(b2fix)
    return m


def kernel(**inputs):
    inputs = {k: np.asarray(v) for k, v in inputs.items()}
    scalars = tuple(float(inputs[k]) for k in
                    ("alpha", "delta", "gamma", "beta", "alpha_new",
                     "delta_new"))
    use_b2fix = bool(np.abs(np.concatenate(
        [inputs["om_b2"], inputs["oc_b2"]])).max() > 0)
    use_lnew = not (scalars[4] == scalars[0] and scalars[5] == scalars[1])

    key = (scalars, use_b2fix, use_lnew)
    if key not in _CACHE:
        _CACHE[key] = _build(scalars, use_b2fix, use_lnew)
    nc = _CACHE[key]

    in_maps = [_host_prep(inputs, r) for r in range(NCORES)]
    res = bass_utils.run_bass_kernel_spmd(nc, in_maps,
                                          core_ids=list(range(NCORES)))
    return _assemble(res.results, scalars, use_lnew)


def _assemble(results, scalars, use_lnew):
    f32 = np.float32
    x_hat = np.zeros([B, T, N, MD], f32)
    logit = np.zeros([B, T, N, N], f32)
    w = np.zeros([B, T, N, N], f32)
    l_new = np.zeros([B, T, N, N], f32) if use_lnew else logit
    zm = np.zeros([B, T, N, L], f32)
    zc = np.zeros([B, T, N, L], f32)
    mu_m = np.zeros([B, N, L], f32)
    lv_m = np.zeros([B, N, L], f32)
    mu_c = np.zeros([B, N, L], f32)
    lv_c = np.zeros([B, N, L], f32)
    for r in range(NCORES):
        o = results[r]
        for s in range(S):
            b = r * S + s
            logit[b] = o["logit"][s]
            w[b] = o["wout"][s]
            if use_lnew:
                l_new[b] = o["lnew"][s]
            x_hat[b] = o["xhatT"][s].transpose(0, 2, 1)
            ztr = o["zt"][:, s]                    # [T, ZD, N]
            zm[b] = ztr[:, 0:L, :].transpose(0, 2, 1)
            zc[b] = ztr[:, L:ZD, :].transpose(0, 2, 1)
            mu_m[b] = o["muT"][s, 0:L].T
            mu_c[b] = o["muT"][s, L:ZD].T
            lv_m[b] = o["lvT"][s, 0:L].T
            lv_c[b] = o["lvT"][s, L:ZD].T
    return (x_hat, logit, w, zm, zc, mu_m, lv_m, mu_c, lv_c, w, l_new)


# revision 30
# speedup vs baseline: 1.0439x; 1.0439x over previous
"""Trainium2 Bass kernel for nn_CLGODE (graph coupled latent graph ODE).

Strategy (pure data parallel, 2 samples per core x 8 cores):
  - Host precomputes: normalized-adjacency pieces (a_hat, d, d*x0), covariate
    embeddings folded with the time term into a per-(sample,eval) bias table,
    h*W2 folded weights per (sample,interval), and all small constants.
  - Device (per core, feature-major state zT [128, 2, 400]):
      encoder GCN + 2 MLP heads -> z0
      RK4 ODE: 7 intervals x 8 steps x 4 stages; stage MLPs on PE,
      relu+bias fused PSUM->SBUF on ACT/DVE, RK4 combines as fused
      scalar_tensor_tensor ops on DVE.
      decoder: score = zc zc^T via PE (symmetric => row blocks only),
      softplus/affine on ACT/DVE, x_hat MLP.
  - Outputs written feature-major where needed; host does the cheap
    final transposes (x_hat, zm, zc, mu/lv). logit/w/l_new are symmetric
    and written directly.
"""
import numpy as np
import ml_dtypes

import concourse.bass as bass
import concourse.bacc as bacc
import concourse.mybir as mybir
import concourse.tile as tile
import concourse.bass_utils as bass_utils

F32 = mybir.dt.float32
BF16 = mybir.dt.bfloat16
AF = mybir.ActivationFunctionType
OP = mybir.AluOpType


def _patch_act_tables():
    """Prefer the table set holding exp+ln+relu+identity+copy so the whole
    kernel needs a single ACT_TABLE_LOAD instead of thrashing per call."""
    import concourse.bacc as _bm
    if getattr(_bm, "_act_tables_patched", False):
        return
    orig = _bm.get_activation_tables

    # IMPORTANT: the first entry of the table list is treated as the
    # boot-resident set (no load emitted before its first use), so the
    # original order must be preserved — reordering makes ops run against
    # the wrong resident tables (observed: Ln returning garbage). With the
    # original order, Exp loads exp_and_others and Ln loads natural_log;
    # the batched score phase keeps that to ~2 loads per time-point.
    _bm._act_tables_patched = True

# problem constants (hardcoded per contract)
B, N, MD, L, H, CE, NCOV, STEPS, T = 16, 400, 8, 64, 128, 8, 2, 8, 8
NIV = T - 1                       # 7 intervals
ZD = 2 * L                        # 128 latent dims
NCORES = 8
S = B // NCORES                   # 2 samples per core
JB = [(0, 128), (128, 128), (256, 128), (384, 16)]  # node row blocks
RK_C = [0.5, 0.5, 1.0]            # stage input coefficients
RK_W = [1.0 / 6, 1.0 / 3, 1.0 / 3, 1.0 / 6]

_CACHE = {}


def _beff_col(iv, k, st, path, s):
    return (((iv * STEPS + k) * 4 + st) * 2 + path) * 2 + s


def _build(scalars, use_b2fix, use_lnew):
    """Build + compile the (SPMD, per-core) Bass program once."""
    alpha, delta, gamma, beta, alpha_new, delta_new = scalars
    nc = bacc.Bacc("TRN2", target_bir_lowering=False, debug=False,
                   num_devices=NCORES)

    def din(name, shape):
        return nc.dram_tensor(name, shape, F32, kind="ExternalInput").ap()

    def dout(name, shape):
        return nc.dram_tensor(name, shape, F32, kind="ExternalOutput").ap()

    def dinb(name, shape):
        return nc.dram_tensor(name, shape, BF16, kind="ExternalInput").ap()

    ahat_d = din("ahat", [S, N, N])
    u_d = din("u", [S, N, MD])
    dvec_d = din("dvec", [S, N])
    w1z_d = dinb("w1z", [2, ZD, H])
    hbw2_d = dinb("hbw2", [NIV, S, 2, H, L])
    beff_d = din("beff", [ZD, NIV * STEPS * 4 * 4])
    encw1_d = din("encw1", [2, MD, H])
    wmu_d = din("wmu", [2, H, L])
    wlv_d = din("wlv", [2, H, L])
    dmw1_d = din("dmw1", [L, H])
    dmw2_d = din("dmw2", [H, MD])
    encb_d = din("encb", [H, 6])
    ident_d = din("ident", [128, 128])
    masks_d = din("masks", [128, 4 * N])
    if use_b2fix:
        b2fix_d = din("b2fix", [1, NIV * S * ZD])

    logit_d = dout("logit", [S, T, N, N])
    w_d = dout("wout", [S, T, N, N])
    if use_lnew:
        lnew_d = dout("lnew", [S, T, N, N])
    xhatT_d = dout("xhatT", [S, T, MD, N])
    zt_d = dout("zt", [T, S, ZD, N])
    warm_d = dout("warm", [128, 512])
    muT_d = dout("muT", [S, ZD, N])
    lvT_d = dout("lvT", [S, ZD, N])

    with tile.TileContext(nc) as tc:
        from contextlib import ExitStack
        with ExitStack() as ctx:
            const = ctx.enter_context(tc.tile_pool(name="const", bufs=1))
            # ---- constants into SBUF ----
            w1z_sb = const.tile([ZD, 2, H], BF16, name="w1z_sb")
            nc.sync.dma_start(w1z_sb[:], w1z_d.rearrange("p k m -> k p m"))
            hbw2_sb = const.tile([H, NIV, S, 2, L], BF16, name="hbw2_sb")
            nc.sync.dma_start(hbw2_sb[:], hbw2_d.rearrange("i s p k m -> k i s p m"))
            beff_sb = const.tile([ZD, NIV * STEPS * 4 * 4], F32, name="beff_sb")
            nc.sync.dma_start(beff_sb[:], beff_d)
            encw1_sb = const.tile([MD, 2, H], F32, name="encw1_sb")
            nc.sync.dma_start(encw1_sb[:], encw1_d.rearrange("p k m -> k p m"))
            wmu_sb = const.tile([H, 2, L], F32, name="wmu_sb")
            nc.sync.dma_start(wmu_sb[:], wmu_d.rearrange("p k m -> k p m"))
            wlv_sb = const.tile([H, 2, L], F32, name="wlv_sb")
            nc.sync.dma_start(wlv_sb[:], wlv_d.rearrange("p k m -> k p m"))
            dmw1_sb = const.tile([L, H], F32, name="dmw1_sb")
            nc.sync.dma_start(dmw1_sb[:], dmw1_d)
            dmw2_sb = const.tile([H, MD], F32, name="dmw2_sb")
            nc.sync.dma_start(dmw2_sb[:], dmw2_d)
            encb_sb = const.tile([H, 6], F32, name="encb_sb")
            nc.sync.dma_start(encb_sb[:], encb_d)
            ident_sb = const.tile([128, 128], F32, name="ident_sb")
            nc.sync.dma_start(ident_sb[:], ident_d)
            masks_sb = const.tile([128, 4 * N], F32, name="masks_sb")
            nc.sync.dma_start(masks_sb[:], masks_d)
            if use_b2fix:
                b2fix_sb = const.tile([1, NIV * S * ZD], F32, name="b2fix_sb")
                nc.sync.dma_start(b2fix_sb[:], b2fix_d)
                ones_sb = const.tile([1, N], F32, name="ones_sb")
                nc.vector.memset(ones_sb[:], 1.0)

            # ---- persistent z history + work tiles ----
            zhp = ctx.enter_context(tc.tile_pool(name="zhp", bufs=1))
            zh = [zhp.tile([ZD, S, 512], F32, tag=f"zh{t}", name=f"zh{t}")
                  for t in range(T)]
            work = ctx.enter_context(tc.tile_pool(name="work", bufs=2))

            # ---- psum pools: 2+2+2+2 = 8 banks ----
            pAm = ctx.enter_context(tc.tile_pool(name="pAm", bufs=2, space="PSUM"))
            pAc = ctx.enter_context(tc.tile_pool(name="pAc", bufs=2, space="PSUM"))
            pC = ctx.enter_context(tc.tile_pool(name="pC", bufs=1, space="PSUM"))
            pM = ctx.enter_context(tc.tile_pool(name="pM", bufs=2, space="PSUM"))

            stage = ctx.enter_context(tc.tile_pool(name="stage", bufs=3))
            use_gp_lg2 = (gamma * delta + beta) == 0.0
            if use_gp_lg2:
                # w-chain: spu = (g/a)*logit + log1p-part (exact when g*d+b=0)
                csc_sb = const.tile([128, N], F32, name="csc_sb")
                nc.vector.memset(csc_sb[:], float(gamma / alpha))

            # ---- PE warm-up: ~11us dense matmul burst flips the HAM clock
            # gate to K=8/8 before the encoder; consumed via a dummy output.
            wps = pM.tile([128, 512], F32, tag="m", name="wps")
            for i in range(26):
                nc.tensor.matmul(wps[:, 0:512], w1z_sb[:, 0, :],
                                 hbw2_sb[:, 0:2, :, :, :],
                                 start=(i == 0), stop=(i == 25))
            wsb = stage.tile([128, 512], F32, tag="warm", name="wsb")
            nc.scalar.copy(wsb[:], wps[:, 0:512])
            nc.sync.dma_start(warm_d[:], wsb[:])


            # ================= encoder =================
            with tc.tile_pool(name="enc", bufs=1) as enc:
                ahat_sb = []
                for s in range(S):
                    a = enc.tile([128, 4, N], F32, tag=f"ah{s}", name=f"ah{s}")
                    nc.sync.dma_start(
                        a[:, 0:3, :],
                        ahat_d[s, 0:384].rearrange("(k p) n -> p k n", p=128))
                    nc.sync.dma_start(a[0:16, 3, :], ahat_d[s, 384:400])
                    ahat_sb.append(a)
                u_sb = enc.tile([128, S, 4, MD], F32, name="u_sb")
                dv_sb = enc.tile([128, S, 4], F32, name="dv_sb")
                for s in range(S):
                    nc.sync.dma_start(
                        u_sb[:, s, 0:3, :],
                        u_d[s, 0:384].rearrange("(k p) d -> p k d", p=128))
                    nc.sync.dma_start(u_sb[0:16, s, 3, :], u_d[s, 384:400])
                    nc.sync.dma_start(
                        dv_sb[:, s, 0:3],
                        dvec_d[s, 0:384].rearrange("(k p) -> p k", p=128))
                    nc.sync.dma_start(dv_sb[0:16, s, 3], dvec_d[s, 384:400])

                hx_sb = enc.tile([128, S, 4, MD], F32, name="hx_sb")
                hxT_sb = enc.tile([MD, S, 512], F32, name="hxT_sb")
                for s in range(S):
                    for j, (j0, jsz) in enumerate(JB):
                        ps_y = pM.tile([128, 512], F32, tag="m", name="ps_y")
                        for kb, (k0, ksz) in enumerate(JB):
                            nc.tensor.matmul(
                                ps_y[0:jsz, 0:MD],
                                ahat_sb[s][0:ksz, kb, j0:j0 + jsz],
                                u_sb[0:ksz, s, kb, :],
                                start=(kb == 0), stop=(kb == 3))
                        nc.vector.tensor_scalar(
                            hx_sb[0:jsz, s, j, :], ps_y[0:jsz, 0:MD],
                            dv_sb[0:jsz, s, j:j + 1], None, OP.mult)
                        ps_hxT = pM.tile([128, 512], F32, tag="m", name="ps_hxT")
                        nc.tensor.transpose(
                            ps_hxT[0:MD, 0:jsz], hx_sb[0:jsz, s, j, :],
                            ident_sb[0:jsz, 0:jsz])
                        nc.scalar.copy(hxT_sb[:, s, j0:j0 + jsz],
                                       ps_hxT[0:MD, 0:jsz])

                hm_sb = enc.tile([H, S, 512], F32, name="hm_sb")
                hc_sb = enc.tile([H, S, 512], F32, name="hc_sb")
                for s in range(S):
                    ps_hm = pAm.tile([H, 512], F32, tag="a", name="ps_hm")
                    nc.tensor.matmul(ps_hm[:, 0:N], encw1_sb[:, 0, :],
                                     hxT_sb[:, s, 0:N])
                    nc.scalar.activation(hm_sb[:, s, 0:N], ps_hm[:, 0:N],
                                         AF.Relu, bias=encb_sb[:, 0:1])
                    ps_hc = pAc.tile([H, 512], F32, tag="a", name="ps_hc")
                    nc.tensor.matmul(ps_hc[:, 0:N], encw1_sb[:, 1, :],
                                     hxT_sb[:, s, 0:N])
                    nc.scalar.activation(hc_sb[:, s, 0:N], ps_hc[:, 0:N],
                                         AF.Relu, bias=encb_sb[:, 1:2])
                lv_sb = enc.tile([ZD, S, 512], F32, name="lv_sb")
                for s in range(S):
                    ps_mu = pC.tile([ZD, S, 512], F32, tag="c", name="ps_mu")
                    nc.tensor.matmul(ps_mu[0:L, s, 0:N], wmu_sb[:, 0, :],
                                     hm_sb[:, s, 0:N])
                    nc.tensor.matmul(ps_mu[L:ZD, s, 0:N], wmu_sb[:, 1, :],
                                     hc_sb[:, s, 0:N])
                    nc.scalar.activation(zh[0][:, s, 0:N], ps_mu[:, s, 0:N],
                                         AF.Identity, bias=encb_sb[:, 2:3])
                    nc.sync.dma_start(muT_d[s], zh[0][:, s, 0:N])
                    ps_lv = pM.tile([ZD, 512], F32, tag="m", name="ps_lv")
                    nc.tensor.matmul(ps_lv[0:L, 0:N], wlv_sb[:, 0, :],
                                     hm_sb[:, s, 0:N])
                    nc.tensor.matmul(ps_lv[L:ZD, 0:N], wlv_sb[:, 1, :],
                                     hc_sb[:, s, 0:N])
                    nc.scalar.activation(lv_sb[:, s, 0:N], ps_lv[:, 0:N],
                                         AF.Identity, bias=encb_sb[:, 3:4])
                    nc.sync.dma_start(lvT_d[s], lv_sb[:, s, 0:N])

            # ================= decode/score phase =================
            # Batched per t: all exps back-to-back, then all lns, so the ACT
            # table set switches only twice per t. Post-ln arithmetic runs on
            # GpSimd using the logit tensor (score = (logit+a*d)/a).
            def score_phase(t):
                lgt, lg2t, spet, spt = {}, {}, {}, {}
                zbt = {}
                for s in range(S):
                    zb = stage.tile([ZD, 512], BF16, tag=f"zb{s}", bufs=2,
                                    name="zb")
                    nc.vector.tensor_copy(zb[:, 0:N], zh[t][:, s, 0:N])
                    zbt[s] = zb
                for s in range(S):
                    for j, (j0, jsz) in enumerate(JB):
                        ps_sc = pM.tile([128, 512], F32, tag="m", name="ps_sc")
                        nc.tensor.matmul(ps_sc[0:jsz, 0:N],
                                         zbt[s][L:ZD, j0:j0 + jsz],
                                         zbt[s][L:ZD, 0:N])
                        lg = stage.tile([128, N], F32, tag=f"lg{s}{j}",
                                        bufs=1, name="lg")
                        nc.scalar.activation(
                            lg[0:jsz, :], ps_sc[0:jsz, 0:N], AF.Copy,
                            bias=float(-alpha * delta), scale=float(alpha))
                        nc.sync.dma_start(logit_d[s, t, j0:j0 + jsz, :],
                                          lg[0:jsz, :])
                        lg2 = stage.tile([128, N], F32, tag=f"lg2{s}{j}",
                                         bufs=1, name="lg2")
                        if use_gp_lg2:
                            nc.gpsimd.tensor_tensor(
                                lg2[0:jsz, :], lg[0:jsz, :],
                                csc_sb[0:jsz, :], OP.mult)
                        else:
                            nc.vector.tensor_scalar(
                                lg2[0:jsz, :], ps_sc[0:jsz, 0:N],
                                float(gamma), float(beta), OP.mult, OP.add)
                        if use_lnew:
                            lnw = stage.tile([128, N], F32, tag="lnw",
                                             name="lnw")
                            nc.vector.tensor_scalar(
                                lnw[0:jsz, :], ps_sc[0:jsz, 0:N],
                                float(alpha_new), float(-alpha_new * delta_new),
                                OP.mult, OP.add)
                            nc.sync.dma_start(lnew_d[s, t, j0:j0 + jsz, :],
                                              lnw[0:jsz, :])
                        # stable softplus(x)=x+log1p(exp(-x)), x=g*s+b >= 0
                        spe = stage.tile([128, N], F32, tag=f"spe{s}{j}",
                                         bufs=1, name="spe")
                        exp_inst = nc.scalar.activation(
                            spe[0:jsz, :], ps_sc[0:jsz, 0:N],
                            AF.Exp, bias=float(-beta), scale=float(-gamma))
                        lgt[(s, j)] = lg
                        lg2t[(s, j)] = lg2
                        spet[(s, j)] = spe
                from concourse.tile import add_dep_helper
                for s in range(S):
                    for j, (j0, jsz) in enumerate(JB):
                        sp = stage.tile([128, N], F32, tag=f"sp{s}{j}",
                                        bufs=1, name="sp")
                        ln_inst = nc.scalar.activation(
                            sp[0:jsz, :], spet[(s, j)][0:jsz, :],
                            AF.Ln, bias=1.0)
                        # whole ln batch after whole exp batch: 2 ACT table
                        # switches per t instead of ~5
                        add_dep_helper(ln_inst.ins, exp_inst.ins, sync=True,
                                       reason="batch ln after exp")
                        spt[(s, j)] = sp
                for s in range(S):
                    for j, (j0, jsz) in enumerate(JB):
                        spu = stage.tile([128, N], F32, tag="spu", name="spu")
                        nc.gpsimd.tensor_tensor(
                            spu[0:jsz, :], lg2t[(s, j)][0:jsz, :],
                            spt[(s, j)][0:jsz, :], OP.add)
                        wt = stage.tile([128, N], F32, tag="wt", name="wt")
                        nc.gpsimd.tensor_tensor(
                            wt[0:jsz, :], spu[0:jsz, :],
                            masks_sb[0:jsz, j * N:(j + 1) * N], OP.mult)
                        nc.sync.dma_start(w_d[s, t, j0:j0 + jsz, :],
                                          wt[0:jsz, :])
                for s in range(S):
                    # x_hat head
                    ps_hd = pM.tile([128, 512], F32, tag="m", name="ps_hd")
                    nc.tensor.matmul(ps_hd[:, 0:N], dmw1_sb[:],
                                     zh[t][0:L, s, 0:N])
                    hd = stage.tile([H, N], F32, tag="hd", name="hd")
                    nc.scalar.activation(hd[:], ps_hd[:, 0:N], AF.Relu,
                                         bias=encb_sb[:, 4:5])
                    ps_xh = pM.tile([128, 512], F32, tag="m", name="ps_xh")
                    nc.tensor.matmul(ps_xh[0:MD, 0:N], dmw2_sb[:], hd[:])
                    xh = stage.tile([MD, N], F32, tag="xh", name="xh")
                    nc.scalar.activation(xh[:], ps_xh[0:MD, 0:N], AF.Identity,
                                         bias=encb_sb[0:MD, 5:6])
                    nc.sync.dma_start(xhatT_d[s, t], xh[:])
                    nc.sync.dma_start(zt_d[t, s], zh[t][:, s, 0:N])

            # ================= one RK4 step =================
            def ode_step(iv, k, z_in, z_out):
                acc = z_in
                zbf = work.tile([ZD, S, 512], BF16, tag="zbf", name="zbf")
                nc.vector.tensor_copy(zbf[:, :, 0:N], z_in[:, :, 0:N])
                ytile = zbf
                for st in range(4):
                    ev = (iv * STEPS + k) * 4 + st
                    psA = [[pAm.tile([H, 512], F32, tag="a", name="psAm")
                            for s in range(S)],
                           [pAc.tile([H, 512], F32, tag="a", name="psAc")
                            for s in range(S)]]
                    for p in range(2):
                        # pacemaker: dep-free filler that issues as soon as
                        # the psum slot frees, keeping the PE HAM clock gate
                        # warm through the relu/STT wait; overwritten by the
                        # real matmul below (start=True).
                        nc.tensor.matmul(psA[p][0][:, 0:512],
                                         w1z_sb[:, p, :],
                                         hbw2_sb[:, 0:2, :, :, :])
                    for p in range(2):
                        for s in range(S):
                            nc.tensor.matmul(psA[p][s][:, 0:N],
                                             w1z_sb[:, p, :],
                                             ytile[:, s, 0:N])
                    h1 = [work.tile([H, S, 512], BF16, tag="h1m", name="h1m"),
                          work.tile([H, S, 512], BF16, tag="h1c", name="h1c")]
                    for p in range(2):
                        for s in range(S):
                            r = st * 4 + p * 2 + s
                            bcol = _beff_col(iv, k, st, p, s)
                            bias_ap = beff_sb[:, bcol:bcol + 1]
                            if r % 8 == 3:   # 2 of 16 per step on DVE
                                nc.vector.tensor_scalar(
                                    h1[p][:, s, 0:N], psA[p][s][:, 0:N],
                                    bias_ap, 0.0, OP.add, OP.max)
                            else:
                                nc.scalar.activation(
                                    h1[p][:, s, 0:N], psA[p][s][:, 0:N],
                                    AF.Relu, bias=bias_ap)
                    psC = pC.tile([ZD, S, 512], F32, tag="c", name="psC")
                    nc.tensor.matmul(psC[:, 0, 0:512], w1z_sb[:, 1, :],
                                     hbw2_sb[:, 2:4, :, :, :])
                    fix = use_b2fix and st == 3
                    for s in range(S):
                        if fix:
                            off = (iv * S + s) * ZD
                            nc.tensor.matmul(
                                psC[:, s, 0:N],
                                b2fix_sb[:, off:off + ZD],
                                ones_sb[:, 0:N], start=True, stop=False)
                        for p in range(2):
                            nc.tensor.matmul(
                                psC[p * L:(p + 1) * L, s, 0:N],
                                hbw2_sb[:, iv, s, p, :],
                                h1[p][:, s, 0:N],
                                start=not fix, stop=True)
                    if st < 3:
                        ynew = work.tile([ZD, S, 512], BF16, tag="y", name="y")
                        for s in range(S):
                            nc.vector.scalar_tensor_tensor(
                                ynew[:, s, 0:N], psC[:, s, 0:N], RK_C[st],
                                z_in[:, s, 0:N], OP.mult, OP.add)
                        accnew = work.tile([ZD, S, 512], F32, tag="acc",
                                           name="acc")
                        nc.vector.scalar_tensor_tensor(
                            accnew[:, :, 0:N], psC[:, :, 0:N], RK_W[st],
                            acc[:, :, 0:N], OP.mult, OP.add)
                        ytile = ynew
                        acc = accnew
                    else:
                        nc.vector.scalar_tensor_tensor(
                            z_out[:, :, 0:N], psC[:, :, 0:N], RK_W[st],
                            acc[:, :, 0:N], OP.mult, OP.add)

            # ================= main schedule =================
            score_phase(0)
            for iv in range(NIV):
                z_in = zh[iv]
                for k in range(STEPS):
                    if k == STEPS - 1:
                        z_out = zh[iv + 1]
                    else:
                        z_out = work.tile([ZD, S, 512], F32, tag="zw",
                                          name="zw")
                    ode_step(iv, k, z_in, z_out)
                    z_in = z_out
                score_phase(iv + 1)

    _patch_act_tables()
    nc.compile()
    return nc


def _host_prep(inputs, core):
    """Per-core input arrays (numpy) for in_maps."""
    f32 = np.float32
    bsel = [core * S + s for s in range(S)]
    a0 = inputs["a0"][bsel].astype(np.float64)
    x0 = inputs["x0"][bsel].astype(np.float64)
    times = inputs["times"].astype(np.float64)
    eye = np.eye(N)
    ahat = a0 + eye
    d = (ahat.sum(-1) + 1e-8) ** -0.5            # [S,N]
    u = d[..., None] * x0                        # [S,N,MD]

    sex = np.asarray(inputs["sex"]).astype(np.int64)
    site = np.asarray(inputs["site"]).astype(np.int64)
    cov_full = np.concatenate([
        np.asarray(inputs["sex_emb"])[sex],
        np.asarray(inputs["site_emb"])[site],
        np.asarray(inputs["covariates"])], -1).astype(np.float64)  # [B, 18]

    omW1 = np.asarray(inputs["om_W1"], np.float64)
    ocW1 = np.asarray(inputs["oc_W1"], np.float64)
    omW2 = np.asarray(inputs["om_W2"], np.float64)
    ocW2 = np.asarray(inputs["oc_W2"], np.float64)
    omb1 = np.asarray(inputs["om_b1"], np.float64)
    ocb1 = np.asarray(inputs["oc_b1"], np.float64)
    omb2 = np.asarray(inputs["om_b2"], np.float64)
    ocb2 = np.asarray(inputs["oc_b2"], np.float64)
    b2cat = np.concatenate([omb2, ocb2])          # [128]
    W1z = np.stack([omW1[:ZD], ocW1[:ZD]])        # [2,128,128]
    W1cov = np.stack([omW1[ZD:ZD + 18], ocW1[ZD:ZD + 18]])  # [2,18,128]
    W1t = np.stack([omW1[ZD + 18], ocW1[ZD + 18]])          # [2,128]
    b1 = np.stack([omb1, ocb1])

    hbw2 = np.zeros([NIV, S, 2, H, L])
    beff = np.zeros([ZD, NIV * STEPS * 4 * 4])
    b2fix = np.zeros([1, NIV * S * ZD])
    for s in range(S):
        b = bsel[s]
        base = [cov_full[b] @ W1cov[p] + b1[p] for p in range(2)]
        for iv in range(NIV):
            t0 = times[b, iv]
            hh = (times[b, iv + 1] - t0) / STEPS
            hb = hh
            hbw2[iv, s, 0] = hb * omW2
            hbw2[iv, s, 1] = hb * ocW2
            b2fix[0, (iv * S + s) * ZD:(iv * S + s + 1) * ZD] = 6.0 * hb * b2cat
            for k in range(STEPS):
                tk = t0 + k * hh
                toff = [tk, tk + 0.5 * hh, tk + 0.5 * hh, tk + hh]
                for st in range(4):
                    for p in range(2):
                        v = base[p] + toff[st] * W1t[p]
                        if st > 0:
                            # y_st misses c*(hb*b2cat): pre-act correction
                            v = v + (RK_C[st - 1] * hb) * (b2cat @ W1z[p])
                        beff[:, _beff_col(iv, k, st, p, s)] = v

    masks = np.ones([128, 4 * N], f32)
    for j, (j0, jsz) in enumerate(JB):
        for p in range(jsz):
            masks[p, j * N + j0 + p] = 0.0

    def c(a):
        return np.ascontiguousarray(a, dtype=f32)

    m = {
        "ahat": c(ahat), "u": c(u), "dvec": c(d),
        "w1z": np.ascontiguousarray(W1z, dtype=ml_dtypes.bfloat16),
        "hbw2": np.ascontiguousarray(hbw2, dtype=ml_dtypes.bfloat16),
        "beff": c(beff),
        "encw1": c(np.stack([inputs["em_W1"], inputs["ec_W1"]])),
        "wmu": c(np.stack([inputs["em_Wmu"], inputs["ec_Wmu"]])),
        "wlv": c(np.stack([inputs["em_Wlv"], inputs["ec_Wlv"]])),
        "dmw1": c(inputs["dm_W1"]), "dmw2": c(inputs["dm_W2"]),
        "ident": c(np.eye(128)),
        "masks": masks,
    }
    encb = np.zeros([H, 6])
    encb[:, 0] = inputs["em_b1"]
    encb[:, 1] = inputs["ec_b1"]
    encb[:, 2] = np.concatenate([inputs["em_bmu"], inputs["ec_bmu"]])
    encb[:, 3] = np.concatenate([inputs["em_blv"], inputs["ec_blv"]])
    encb[:, 4] = inputs["dm_b1"]
    encb[0:MD, 5] = inputs["dm_b2"]
    m["encb"] = c(encb)
    if np.abs(b2cat).max() > 0:
        m["b2fix"] = c(b2fix)
    return m


def kernel(**inputs):
    inputs = {k: np.asarray(v) for k, v in inputs.items()}
    scalars = tuple(float(inputs[k]) for k in
                    ("alpha", "delta", "gamma", "beta", "alpha_new",
                     "delta_new"))
    use_b2fix = bool(np.abs(np.concatenate(
        [inputs["om_b2"], inputs["oc_b2"]])).max() > 0)
    use_lnew = not (scalars[4] == scalars[0] and scalars[5] == scalars[1])

    key = (scalars, use_b2fix, use_lnew)
    if key not in _CACHE:
        _CACHE[key] = _build(scalars, use_b2fix, use_lnew)
    nc = _CACHE[key]

    in_maps = [_host_prep(inputs, r) for r in range(NCORES)]
    res = bass_utils.run_bass_kernel_spmd(nc, in_maps,
                                          core_ids=list(range(NCORES)))
    return _assemble(res.results, scalars, use_lnew)


def _assemble(results, scalars, use_lnew):
    f32 = np.float32
    x_hat = np.zeros([B, T, N, MD], f32)
    logit = np.zeros([B, T, N, N], f32)
    w = np.zeros([B, T, N, N], f32)
    l_new = np.zeros([B, T, N, N], f32) if use_lnew else logit
    zm = np.zeros([B, T, N, L], f32)
    zc = np.zeros([B, T, N, L], f32)
    mu_m = np.zeros([B, N, L], f32)
    lv_m = np.zeros([B, N, L], f32)
    mu_c = np.zeros([B, N, L], f32)
    lv_c = np.zeros([B, N, L], f32)
    for r in range(NCORES):
        o = results[r]
        for s in range(S):
            b = r * S + s
            logit[b] = o["logit"][s]
            w[b] = o["wout"][s]
            if use_lnew:
                l_new[b] = o["lnew"][s]
            x_hat[b] = o["xhatT"][s].transpose(0, 2, 1)
            ztr = o["zt"][:, s]                    # [T, ZD, N]
            zm[b] = ztr[:, 0:L, :].transpose(0, 2, 1)
            zc[b] = ztr[:, L:ZD, :].transpose(0, 2, 1)
            mu_m[b] = o["muT"][s, 0:L].T
            mu_c[b] = o["muT"][s, L:ZD].T
            lv_m[b] = o["lvT"][s, 0:L].T
            lv_c[b] = o["lvT"][s, L:ZD].T
    return (x_hat, logit, w, zm, zc, mu_m, lv_m, mu_c, lv_c, w, l_new)


# revision 31
# speedup vs baseline: 1.1368x; 1.0889x over previous
"""Trainium2 Bass kernel for nn_CLGODE (graph coupled latent graph ODE).

Strategy (pure data parallel, 2 samples per core x 8 cores):
  - Host precomputes: normalized-adjacency pieces (a_hat, d, d*x0), covariate
    embeddings folded with the time term into a per-(sample,eval) bias table,
    h*W2 folded weights per (sample,interval), and all small constants.
  - Device (per core, feature-major state zT [128, 2, 400]):
      encoder GCN + 2 MLP heads -> z0
      RK4 ODE: 7 intervals x 8 steps x 4 stages; stage MLPs on PE,
      relu+bias fused PSUM->SBUF on ACT/DVE, RK4 combines as fused
      scalar_tensor_tensor ops on DVE.
      decoder: score = zc zc^T via PE (symmetric => row blocks only),
      softplus/affine on ACT/DVE, x_hat MLP.
  - Outputs written feature-major where needed; host does the cheap
    final transposes (x_hat, zm, zc, mu/lv). logit/w/l_new are symmetric
    and written directly.
"""
import numpy as np
import ml_dtypes

import concourse.bass as bass
import concourse.bacc as bacc
import concourse.mybir as mybir
import concourse.tile as tile
import concourse.bass_utils as bass_utils

F32 = mybir.dt.float32
BF16 = mybir.dt.bfloat16
AF = mybir.ActivationFunctionType
OP = mybir.AluOpType


def _patch_act_tables():
    """Prefer the table set holding exp+ln+relu+identity+copy so the whole
    kernel needs a single ACT_TABLE_LOAD instead of thrashing per call."""
    import concourse.bacc as _bm
    if getattr(_bm, "_act_tables_patched", False):
        return
    orig = _bm.get_activation_tables

    # IMPORTANT: the first entry of the table list is treated as the
    # boot-resident set (no load emitted before its first use), so the
    # original order must be preserved — reordering makes ops run against
    # the wrong resident tables (observed: Ln returning garbage). With the
    # original order, Exp loads exp_and_others and Ln loads natural_log;
    # the batched score phase keeps that to ~2 loads per time-point.
    _bm._act_tables_patched = True

# problem constants (hardcoded per contract)
B, N, MD, L, H, CE, NCOV, STEPS, T = 16, 400, 8, 64, 128, 8, 2, 8, 8
NIV = T - 1                       # 7 intervals
ZD = 2 * L                        # 128 latent dims
NCORES = 8
S = B // NCORES                   # 2 samples per core
JB = [(0, 128), (128, 128), (256, 128), (384, 16)]  # node row blocks
RK_C = [0.5, 0.5, 1.0]            # stage input coefficients
RK_W = [1.0 / 6, 1.0 / 3, 1.0 / 3, 1.0 / 6]

_CACHE = {}


def _beff_col(iv, k, st, path, s):
    return (((iv * STEPS + k) * 4 + st) * 2 + path) * 2 + s


def _build(scalars, use_b2fix, use_lnew):
    """Build + compile the (SPMD, per-core) Bass program once."""
    alpha, delta, gamma, beta, alpha_new, delta_new = scalars
    nc = bacc.Bacc("TRN2", target_bir_lowering=False, debug=False,
                   num_devices=NCORES)

    def din(name, shape):
        return nc.dram_tensor(name, shape, F32, kind="ExternalInput").ap()

    def dout(name, shape):
        return nc.dram_tensor(name, shape, F32, kind="ExternalOutput").ap()

    def dinb(name, shape):
        return nc.dram_tensor(name, shape, BF16, kind="ExternalInput").ap()

    ahat_d = din("ahat", [S, N, N])
    u_d = din("u", [S, N, MD])
    dvec_d = din("dvec", [S, N])
    w1z_d = dinb("w1z", [2, ZD, H])
    hbw2_d = dinb("hbw2", [NIV, S, 2, H, L])
    beff_d = din("beff", [ZD, NIV * STEPS * 4 * 4])
    encw1_d = din("encw1", [2, MD, H])
    wmu_d = din("wmu", [2, H, L])
    wlv_d = din("wlv", [2, H, L])
    dmw1_d = din("dmw1", [L, H])
    dmw2_d = din("dmw2", [H, MD])
    encb_d = din("encb", [H, 6])
    ident_d = din("ident", [128, 128])
    masks_d = din("masks", [128, 4 * N])
    if use_b2fix:
        b2fix_d = din("b2fix", [1, NIV * S * ZD])

    logit_d = dout("logit", [S, T, N, N])
    w_d = dout("wout", [S, T, N, N])
    if use_lnew:
        lnew_d = dout("lnew", [S, T, N, N])
    xhatT_d = dout("xhatT", [S, T, MD, N])
    zt_d = dout("zt", [T, S, ZD, N])
    warm_d = dout("warm", [128, 512])
    muT_d = dout("muT", [S, ZD, N])
    lvT_d = dout("lvT", [S, ZD, N])

    with tile.TileContext(nc) as tc:
        from contextlib import ExitStack
        with ExitStack() as ctx:
            const = ctx.enter_context(tc.tile_pool(name="const", bufs=1))
            # ---- constants into SBUF ----
            w1z_sb = const.tile([ZD, 2, H], BF16, name="w1z_sb")
            nc.sync.dma_start(w1z_sb[:], w1z_d.rearrange("p k m -> k p m"))
            hbw2_sb = const.tile([H, NIV, S, 2, L], BF16, name="hbw2_sb")
            nc.sync.dma_start(hbw2_sb[:], hbw2_d.rearrange("i s p k m -> k i s p m"))
            beff_sb = const.tile([ZD, NIV * STEPS * 4 * 4], F32, name="beff_sb")
            nc.sync.dma_start(beff_sb[:], beff_d)
            encw1_sb = const.tile([MD, 2, H], F32, name="encw1_sb")
            nc.sync.dma_start(encw1_sb[:], encw1_d.rearrange("p k m -> k p m"))
            wmu_sb = const.tile([H, 2, L], F32, name="wmu_sb")
            nc.sync.dma_start(wmu_sb[:], wmu_d.rearrange("p k m -> k p m"))
            wlv_sb = const.tile([H, 2, L], F32, name="wlv_sb")
            nc.sync.dma_start(wlv_sb[:], wlv_d.rearrange("p k m -> k p m"))
            dmw1_sb = const.tile([L, H], F32, name="dmw1_sb")
            nc.sync.dma_start(dmw1_sb[:], dmw1_d)
            dmw2_sb = const.tile([H, MD], F32, name="dmw2_sb")
            nc.sync.dma_start(dmw2_sb[:], dmw2_d)
            encb_sb = const.tile([H, 6], F32, name="encb_sb")
            nc.sync.dma_start(encb_sb[:], encb_d)
            ident_sb = const.tile([128, 128], F32, name="ident_sb")
            nc.sync.dma_start(ident_sb[:], ident_d)
            masks_sb = const.tile([128, 4 * N], F32, name="masks_sb")
            nc.sync.dma_start(masks_sb[:], masks_d)
            if use_b2fix:
                b2fix_sb = const.tile([1, NIV * S * ZD], F32, name="b2fix_sb")
                nc.sync.dma_start(b2fix_sb[:], b2fix_d)
                ones_sb = const.tile([1, N], F32, name="ones_sb")
                nc.vector.memset(ones_sb[:], 1.0)

            # ---- persistent z history + work tiles ----
            zhp = ctx.enter_context(tc.tile_pool(name="zhp", bufs=1))
            zh = [zhp.tile([ZD, S, 512], F32, tag=f"zh{t}", name=f"zh{t}")
                  for t in range(T)]
            work = ctx.enter_context(tc.tile_pool(name="work", bufs=2))

            # ---- psum pools: 2+2+2+2 = 8 banks ----
            pAm = ctx.enter_context(tc.tile_pool(name="pAm", bufs=2, space="PSUM"))
            pAc = ctx.enter_context(tc.tile_pool(name="pAc", bufs=2, space="PSUM"))
            pC = ctx.enter_context(tc.tile_pool(name="pC", bufs=1, space="PSUM"))
            pM = ctx.enter_context(tc.tile_pool(name="pM", bufs=2, space="PSUM"))

            stage = ctx.enter_context(tc.tile_pool(name="stage", bufs=3))
            use_gp_lg2 = (gamma * delta + beta) == 0.0
            if use_gp_lg2:
                # w-chain: spu = (g/a)*logit + log1p-part (exact when g*d+b=0)
                csc_sb = const.tile([128, N], F32, name="csc_sb")
                nc.vector.memset(csc_sb[:], float(gamma / alpha))

            # ---- PE warm-up: ~11us dense matmul burst flips the HAM clock
            # gate to K=8/8 before the encoder; consumed via a dummy output.
            wps = pM.tile([128, 512], F32, tag="m", name="wps")
            for i in range(26):
                nc.tensor.matmul(wps[:, 0:512], w1z_sb[:, 0, :],
                                 hbw2_sb[:, 0:2, :, :, :],
                                 start=(i == 0), stop=(i == 25))
            wsb = stage.tile([128, 512], F32, tag="warm", name="wsb")
            nc.scalar.copy(wsb[:], wps[:, 0:512])
            nc.sync.dma_start(warm_d[:], wsb[:])


            # ================= encoder =================
            with tc.tile_pool(name="enc", bufs=1) as enc:
                ahat_sb = []
                for s in range(S):
                    a = enc.tile([128, 4, N], F32, tag=f"ah{s}", name=f"ah{s}")
                    nc.sync.dma_start(
                        a[:, 0:3, :],
                        ahat_d[s, 0:384].rearrange("(k p) n -> p k n", p=128))
                    nc.sync.dma_start(a[0:16, 3, :], ahat_d[s, 384:400])
                    ahat_sb.append(a)
                u_sb = enc.tile([128, S, 4, MD], F32, name="u_sb")
                dv_sb = enc.tile([128, S, 4], F32, name="dv_sb")
                for s in range(S):
                    nc.sync.dma_start(
                        u_sb[:, s, 0:3, :],
                        u_d[s, 0:384].rearrange("(k p) d -> p k d", p=128))
                    nc.sync.dma_start(u_sb[0:16, s, 3, :], u_d[s, 384:400])
                    nc.sync.dma_start(
                        dv_sb[:, s, 0:3],
                        dvec_d[s, 0:384].rearrange("(k p) -> p k", p=128))
                    nc.sync.dma_start(dv_sb[0:16, s, 3], dvec_d[s, 384:400])

                hx_sb = enc.tile([128, S, 4, MD], F32, name="hx_sb")
                hxT_sb = enc.tile([MD, S, 512], F32, name="hxT_sb")
                for s in range(S):
                    for j, (j0, jsz) in enumerate(JB):
                        ps_y = pM.tile([128, 512], F32, tag="m", name="ps_y")
                        for kb, (k0, ksz) in enumerate(JB):
                            nc.tensor.matmul(
                                ps_y[0:jsz, 0:MD],
                                ahat_sb[s][0:ksz, kb, j0:j0 + jsz],
                                u_sb[0:ksz, s, kb, :],
                                start=(kb == 0), stop=(kb == 3))
                        nc.vector.tensor_scalar(
                            hx_sb[0:jsz, s, j, :], ps_y[0:jsz, 0:MD],
                            dv_sb[0:jsz, s, j:j + 1], None, OP.mult)
                        ps_hxT = pM.tile([128, 512], F32, tag="m", name="ps_hxT")
                        nc.tensor.transpose(
                            ps_hxT[0:MD, 0:jsz], hx_sb[0:jsz, s, j, :],
                            ident_sb[0:jsz, 0:jsz])
                        nc.scalar.copy(hxT_sb[:, s, j0:j0 + jsz],
                                       ps_hxT[0:MD, 0:jsz])

                hm_sb = enc.tile([H, S, 512], F32, name="hm_sb")
                hc_sb = enc.tile([H, S, 512], F32, name="hc_sb")
                for s in range(S):
                    ps_hm = pAm.tile([H, 512], F32, tag="a", name="ps_hm")
                    nc.tensor.matmul(ps_hm[:, 0:N], encw1_sb[:, 0, :],
                                     hxT_sb[:, s, 0:N])
                    nc.scalar.activation(hm_sb[:, s, 0:N], ps_hm[:, 0:N],
                                         AF.Relu, bias=encb_sb[:, 0:1])
                    ps_hc = pAc.tile([H, 512], F32, tag="a", name="ps_hc")
                    nc.tensor.matmul(ps_hc[:, 0:N], encw1_sb[:, 1, :],
                                     hxT_sb[:, s, 0:N])
                    nc.scalar.activation(hc_sb[:, s, 0:N], ps_hc[:, 0:N],
                                         AF.Relu, bias=encb_sb[:, 1:2])
                lv_sb = enc.tile([ZD, S, 512], F32, name="lv_sb")
                for s in range(S):
                    ps_mu = pC.tile([ZD, S, 512], F32, tag="c", name="ps_mu")
                    nc.tensor.matmul(ps_mu[0:L, s, 0:N], wmu_sb[:, 0, :],
                                     hm_sb[:, s, 0:N])
                    nc.tensor.matmul(ps_mu[L:ZD, s, 0:N], wmu_sb[:, 1, :],
                                     hc_sb[:, s, 0:N])
                    nc.scalar.activation(zh[0][:, s, 0:N], ps_mu[:, s, 0:N],
                                         AF.Identity, bias=encb_sb[:, 2:3])
                    nc.sync.dma_start(muT_d[s], zh[0][:, s, 0:N])
                    ps_lv = pM.tile([ZD, 512], F32, tag="m", name="ps_lv")
                    nc.tensor.matmul(ps_lv[0:L, 0:N], wlv_sb[:, 0, :],
                                     hm_sb[:, s, 0:N])
                    nc.tensor.matmul(ps_lv[L:ZD, 0:N], wlv_sb[:, 1, :],
                                     hc_sb[:, s, 0:N])
                    nc.scalar.activation(lv_sb[:, s, 0:N], ps_lv[:, 0:N],
                                         AF.Identity, bias=encb_sb[:, 3:4])
                    nc.sync.dma_start(lvT_d[s], lv_sb[:, s, 0:N])

            # ================= decode/score phase =================
            # Batched per t: all exps back-to-back, then all lns, so the ACT
            # table set switches only twice per t. Post-ln arithmetic runs on
            # GpSimd using the logit tensor (score = (logit+a*d)/a).
            def score_phase(t):
                lgt, lg2t, spet, spt = {}, {}, {}, {}
                zbt = {}
                for s in range(S):
                    zb = stage.tile([ZD, 512], BF16, tag=f"zb{s}", bufs=2,
                                    name="zb")
                    nc.vector.tensor_copy(zb[:, 0:N], zh[t][:, s, 0:N])
                    zbt[s] = zb
                for s in range(S):
                    for j, (j0, jsz) in enumerate(JB):
                        ps_sc = pM.tile([128, 512], F32, tag="m", name="ps_sc")
                        nc.tensor.matmul(ps_sc[0:jsz, 0:N],
                                         zbt[s][L:ZD, j0:j0 + jsz],
                                         zbt[s][L:ZD, 0:N])
                        lg = stage.tile([128, N], F32, tag=f"lg{s}{j}",
                                        bufs=1, name="lg")
                        nc.scalar.activation(
                            lg[0:jsz, :], ps_sc[0:jsz, 0:N], AF.Copy,
                            bias=float(-alpha * delta), scale=float(alpha))
                        nc.sync.dma_start(logit_d[s, t, j0:j0 + jsz, :],
                                          lg[0:jsz, :])
                        lg2 = stage.tile([128, N], F32, tag=f"lg2{s}{j}",
                                         bufs=1, name="lg2")
                        if use_gp_lg2:
                            nc.gpsimd.tensor_tensor(
                                lg2[0:jsz, :], lg[0:jsz, :],
                                csc_sb[0:jsz, :], OP.mult)
                        else:
                            nc.vector.tensor_scalar(
                                lg2[0:jsz, :], ps_sc[0:jsz, 0:N],
                                float(gamma), float(beta), OP.mult, OP.add)
                        if use_lnew:
                            lnw = stage.tile([128, N], F32, tag="lnw",
                                             name="lnw")
                            nc.vector.tensor_scalar(
                                lnw[0:jsz, :], ps_sc[0:jsz, 0:N],
                                float(alpha_new), float(-alpha_new * delta_new),
                                OP.mult, OP.add)
                            nc.sync.dma_start(lnew_d[s, t, j0:j0 + jsz, :],
                                              lnw[0:jsz, :])
                        # stable softplus(x)=x+log1p(exp(-x)), x=g*s+b >= 0
                        spe = stage.tile([128, N], F32, tag=f"spe{s}{j}",
                                         bufs=1, name="spe")
                        exp_inst = nc.scalar.activation(
                            spe[0:jsz, :], ps_sc[0:jsz, 0:N],
                            AF.Exp, bias=float(-beta), scale=float(-gamma))
                        lgt[(s, j)] = lg
                        lg2t[(s, j)] = lg2
                        spet[(s, j)] = spe
                from concourse.tile import add_dep_helper
                for s in range(S):
                    for j, (j0, jsz) in enumerate(JB):
                        sp = stage.tile([128, N], F32, tag=f"sp{s}{j}",
                                        bufs=1, name="sp")
                        ln_inst = nc.scalar.activation(
                            sp[0:jsz, :], spet[(s, j)][0:jsz, :],
                            AF.Ln, bias=1.0)
                        # whole ln batch after whole exp batch: 2 ACT table
                        # switches per t instead of ~5
                        add_dep_helper(ln_inst.ins, exp_inst.ins, sync=True,
                                       reason="batch ln after exp")
                        spt[(s, j)] = sp
                for s in range(S):
                    for j, (j0, jsz) in enumerate(JB):
                        spu = stage.tile([128, N], F32, tag="spu", name="spu")
                        nc.gpsimd.tensor_tensor(
                            spu[0:jsz, :], lg2t[(s, j)][0:jsz, :],
                            spt[(s, j)][0:jsz, :], OP.add)
                        wt = stage.tile([128, N], F32, tag="wt", name="wt")
                        nc.gpsimd.tensor_tensor(
                            wt[0:jsz, :], spu[0:jsz, :],
                            masks_sb[0:jsz, j * N:(j + 1) * N], OP.mult)
                        nc.sync.dma_start(w_d[s, t, j0:j0 + jsz, :],
                                          wt[0:jsz, :])
                for s in range(S):
                    # x_hat head
                    ps_hd = pM.tile([128, 512], F32, tag="m", name="ps_hd")
                    nc.tensor.matmul(ps_hd[:, 0:N], dmw1_sb[:],
                                     zh[t][0:L, s, 0:N])
                    hd = stage.tile([H, N], F32, tag="hd", name="hd")
                    nc.scalar.activation(hd[:], ps_hd[:, 0:N], AF.Relu,
                                         bias=encb_sb[:, 4:5])
                    ps_xh = pM.tile([128, 512], F32, tag="m", name="ps_xh")
                    nc.tensor.matmul(ps_xh[0:MD, 0:N], dmw2_sb[:], hd[:])
                    xh = stage.tile([MD, N], F32, tag="xh", name="xh")
                    nc.scalar.activation(xh[:], ps_xh[0:MD, 0:N], AF.Identity,
                                         bias=encb_sb[0:MD, 5:6])
                    nc.sync.dma_start(xhatT_d[s, t], xh[:])
                    nc.sync.dma_start(zt_d[t, s], zh[t][:, s, 0:N])

            # ================= one RK4 step =================
            def ode_step(iv, k, z_in, z_out):
                acc = z_in
                zbf = work.tile([ZD, S, 512], BF16, tag="zbf", name="zbf")
                nc.vector.tensor_copy(zbf[:, :, 0:N], z_in[:, :, 0:N])
                ytile = zbf
                for st in range(4):
                    ev = (iv * STEPS + k) * 4 + st
                    psA = [[pAm.tile([H, 512], F32, tag="a", name="psAm")
                            for s in range(S)],
                           [pAc.tile([H, 512], F32, tag="a", name="psAc")
                            for s in range(S)]]
                    for p in range(2):
                        # pacemaker: dep-free filler that issues as soon as
                        # the psum slot frees, keeping the PE HAM clock gate
                        # warm through the relu/STT wait; overwritten by the
                        # real matmul below (start=True).
                        nc.tensor.matmul(psA[p][0][:, 0:512],
                                         w1z_sb[:, p, :],
                                         hbw2_sb[:, 0:2, :, :, :])
                    for p in range(2):
                        for s in range(S):
                            nc.tensor.matmul(psA[p][s][:, 0:N],
                                             w1z_sb[:, p, :],
                                             ytile[:, s, 0:N])
                    h1 = [work.tile([H, S, 512], BF16, tag="h1m", name="h1m"),
                          work.tile([H, S, 512], BF16, tag="h1c", name="h1c")]
                    for p in range(2):
                        for s in range(S):
                            r = st * 4 + p * 2 + s
                            bcol = _beff_col(iv, k, st, p, s)
                            bias_ap = beff_sb[:, bcol:bcol + 1]
                            if r % 8 == 3:   # 2 of 16 per step on DVE
                                nc.vector.tensor_scalar(
                                    h1[p][:, s, 0:N], psA[p][s][:, 0:N],
                                    bias_ap, 0.0, OP.add, OP.max)
                            else:
                                nc.scalar.activation(
                                    h1[p][:, s, 0:N], psA[p][s][:, 0:N],
                                    AF.Relu, bias=bias_ap)
                    psC = pC.tile([ZD, S, 512], F32, tag="c", name="psC")
                    nc.tensor.matmul(psC[:, 0, 0:512], w1z_sb[:, 1, :],
                                     hbw2_sb[:, 2:4, :, :, :])
                    fix = use_b2fix and st == 3
                    for s in range(S):
                        if fix:
                            off = (iv * S + s) * ZD
                            nc.tensor.matmul(
                                psC[:, s, 0:N],
                                b2fix_sb[:, off:off + ZD],
                                ones_sb[:, 0:N], start=True, stop=False)
                        for p in range(2):
                            nc.tensor.matmul(
                                psC[p * L:(p + 1) * L, s, 0:N],
                                hbw2_sb[:, iv, s, p, :],
                                h1[p][:, s, 0:N],
                                start=not fix, stop=True)
                    if st < 3:
                        ynew = work.tile([ZD, S, 512], BF16, tag="y", name="y")
                        for s in range(S):
                            nc.vector.scalar_tensor_tensor(
                                ynew[:, s, 0:N], psC[:, s, 0:N], RK_C[st],
                                z_in[:, s, 0:N], OP.mult, OP.add)
                        accnew = work.tile([ZD, S, 512], F32, tag="acc",
                                           name="acc")
                        nc.vector.scalar_tensor_tensor(
                            accnew[:, :, 0:N], psC[:, :, 0:N], RK_W[st],
                            acc[:, :, 0:N], OP.mult, OP.add)
                        ytile = ynew
                        acc = accnew
                    else:
                        nc.vector.scalar_tensor_tensor(
                            z_out[:, :, 0:N], psC[:, :, 0:N], RK_W[st],
                            acc[:, :, 0:N], OP.mult, OP.add)

            # ================= main schedule =================
            # score_phase(t) is emitted one interval AFTER zh[t] is ready:
            # emission order sets Tile's scheduling priority, so this makes
            # decode work gap-filler behind the current interval's ODE chain
            # instead of preempting its relus on ACT (the relus gate L2/PE).
            for iv in range(NIV):
                z_in = zh[iv]
                for k in range(STEPS):
                    if k == STEPS - 1:
                        z_out = zh[iv + 1]
                    else:
                        z_out = work.tile([ZD, S, 512], F32, tag="zw",
                                          name="zw")
                    ode_step(iv, k, z_in, z_out)
                    z_in = z_out
                score_phase(iv)
            score_phase(NIV)

    _patch_act_tables()
    nc.compile()
    return nc


def _host_prep(inputs, core):
    """Per-core input arrays (numpy) for in_maps."""
    f32 = np.float32
    bsel = [core * S + s for s in range(S)]
    a0 = inputs["a0"][bsel].astype(np.float64)
    x0 = inputs["x0"][bsel].astype(np.float64)
    times = inputs["times"].astype(np.float64)
    eye = np.eye(N)
    ahat = a0 + eye
    d = (ahat.sum(-1) + 1e-8) ** -0.5            # [S,N]
    u = d[..., None] * x0                        # [S,N,MD]

    sex = np.asarray(inputs["sex"]).astype(np.int64)
    site = np.asarray(inputs["site"]).astype(np.int64)
    cov_full = np.concatenate([
        np.asarray(inputs["sex_emb"])[sex],
        np.asarray(inputs["site_emb"])[site],
        np.asarray(inputs["covariates"])], -1).astype(np.float64)  # [B, 18]

    omW1 = np.asarray(inputs["om_W1"], np.float64)
    ocW1 = np.asarray(inputs["oc_W1"], np.float64)
    omW2 = np.asarray(inputs["om_W2"], np.float64)
    ocW2 = np.asarray(inputs["oc_W2"], np.float64)
    omb1 = np.asarray(inputs["om_b1"], np.float64)
    ocb1 = np.asarray(inputs["oc_b1"], np.float64)
    omb2 = np.asarray(inputs["om_b2"], np.float64)
    ocb2 = np.asarray(inputs["oc_b2"], np.float64)
    b2cat = np.concatenate([omb2, ocb2])          # [128]
    W1z = np.stack([omW1[:ZD], ocW1[:ZD]])        # [2,128,128]
    W1cov = np.stack([omW1[ZD:ZD + 18], ocW1[ZD:ZD + 18]])  # [2,18,128]
    W1t = np.stack([omW1[ZD + 18], ocW1[ZD + 18]])          # [2,128]
    b1 = np.stack([omb1, ocb1])

    hbw2 = np.zeros([NIV, S, 2, H, L])
    beff = np.zeros([ZD, NIV * STEPS * 4 * 4])
    b2fix = np.zeros([1, NIV * S * ZD])
    for s in range(S):
        b = bsel[s]
        base = [cov_full[b] @ W1cov[p] + b1[p] for p in range(2)]
        for iv in range(NIV):
            t0 = times[b, iv]
            hh = (times[b, iv + 1] - t0) / STEPS
            hb = hh
            hbw2[iv, s, 0] = hb * omW2
            hbw2[iv, s, 1] = hb * ocW2
            b2fix[0, (iv * S + s) * ZD:(iv * S + s + 1) * ZD] = 6.0 * hb * b2cat
            for k in range(STEPS):
                tk = t0 + k * hh
                toff = [tk, tk + 0.5 * hh, tk + 0.5 * hh, tk + hh]
                for st in range(4):
                    for p in range(2):
                        v = base[p] + toff[st] * W1t[p]
                        if st > 0:
                            # y_st misses c*(hb*b2cat): pre-act correction
                            v = v + (RK_C[st - 1] * hb) * (b2cat @ W1z[p])
                        beff[:, _beff_col(iv, k, st, p, s)] = v

    masks = np.ones([128, 4 * N], f32)
    for j, (j0, jsz) in enumerate(JB):
        for p in range(jsz):
            masks[p, j * N + j0 + p] = 0.0

    def c(a):
        return np.ascontiguousarray(a, dtype=f32)

    m = {
        "ahat": c(ahat), "u": c(u), "dvec": c(d),
        "w1z": np.ascontiguousarray(W1z, dtype=ml_dtypes.bfloat16),
        "hbw2": np.ascontiguousarray(hbw2, dtype=ml_dtypes.bfloat16),
        "beff": c(beff),
        "encw1": c(np.stack([inputs["em_W1"], inputs["ec_W1"]])),
        "wmu": c(np.stack([inputs["em_Wmu"], inputs["ec_Wmu"]])),
        "wlv": c(np.stack([inputs["em_Wlv"], inputs["ec_Wlv"]])),
        "dmw1": c(inputs["dm_W1"]), "dmw2": c(inputs["dm_W2"]),
        "ident": c(np.eye(128)),
        "masks": masks,
    }
    encb = np.zeros([H, 6])
    encb[:, 0] = inputs["em_b1"]
    encb[:, 1] = inputs["ec_b1"]
    encb[:, 2] = np.concatenate([inputs["em_bmu"], inputs["ec_bmu"]])
    encb[:, 3] = np.concatenate([inputs["em_blv"], inputs["ec_blv"]])
    encb[:, 4] = inputs["dm_b1"]
    encb[0:MD, 5] = inputs["dm_b2"]
    m["encb"] = c(encb)
    if np.abs(b2cat).max() > 0:
        m["b2fix"] = c(b2fix)
    return m


def kernel(**inputs):
    inputs = {k: np.asarray(v) for k, v in inputs.items()}
    scalars = tuple(float(inputs[k]) for k in
                    ("alpha", "delta", "gamma", "beta", "alpha_new",
                     "delta_new"))
    use_b2fix = bool(np.abs(np.concatenate(
        [inputs["om_b2"], inputs["oc_b2"]])).max() > 0)
    use_lnew = not (scalars[4] == scalars[0] and scalars[5] == scalars[1])

    key = (scalars, use_b2fix, use_lnew)
    if key not in _CACHE:
        _CACHE[key] = _build(scalars, use_b2fix, use_lnew)
    nc = _CACHE[key]

    in_maps = [_host_prep(inputs, r) for r in range(NCORES)]
    res = bass_utils.run_bass_kernel_spmd(nc, in_maps,
                                          core_ids=list(range(NCORES)))
    return _assemble(res.results, scalars, use_lnew)


def _assemble(results, scalars, use_lnew):
    f32 = np.float32
    x_hat = np.zeros([B, T, N, MD], f32)
    logit = np.zeros([B, T, N, N], f32)
    w = np.zeros([B, T, N, N], f32)
    l_new = np.zeros([B, T, N, N], f32) if use_lnew else logit
    zm = np.zeros([B, T, N, L], f32)
    zc = np.zeros([B, T, N, L], f32)
    mu_m = np.zeros([B, N, L], f32)
    lv_m = np.zeros([B, N, L], f32)
    mu_c = np.zeros([B, N, L], f32)
    lv_c = np.zeros([B, N, L], f32)
    for r in range(NCORES):
        o = results[r]
        for s in range(S):
            b = r * S + s
            logit[b] = o["logit"][s]
            w[b] = o["wout"][s]
            if use_lnew:
                l_new[b] = o["lnew"][s]
            x_hat[b] = o["xhatT"][s].transpose(0, 2, 1)
            ztr = o["zt"][:, s]                    # [T, ZD, N]
            zm[b] = ztr[:, 0:L, :].transpose(0, 2, 1)
            zc[b] = ztr[:, L:ZD, :].transpose(0, 2, 1)
            mu_m[b] = o["muT"][s, 0:L].T
            mu_c[b] = o["muT"][s, L:ZD].T
            lv_m[b] = o["lvT"][s, 0:L].T
            lv_c[b] = o["lvT"][s, L:ZD].T
    return (x_hat, logit, w, zm, zc, mu_m, lv_m, mu_c, lv_c, w, l_new)
